# revision 1
# baseline (speedup 1.0000x reference)
"""CosineAttention Trainium2 kernel.

reference:
    xn  = x / max(||x_i||, eps)        # row-normalize
    sim = xn @ xn.T                    # [N, N]
    out = sigmoid(sim @ x)             # [N, D]

Strategy (8 cores, 1-D row-parallel):
    Each core owns ROWS = N/8 output rows.  Per core, stream all of x once
    in blocks of NB rows.  Per block J:
      - round block to f32r (DVE copy) for full-rate PE matmuls
      - PE-transpose the block into xT (contraction layout)
      - MM1: G^T[J, :] = x[J] @ x_loc^T   (f32r matmuls, N=512 free dim)
      - scale rows of G^T by 1/||x_n|| during PSUM->SBUF copy (ACT engine)
      - MM2: P += (G^T[J])^T @ x[J]       (PSUM accumulate + DVE add into SBUF)
    f32r (TF32-like) matmuls run at full PE rate with ~1e-3 rel error.
    Finally out = sigmoid(P * (1/||x_loc||)) fused on the ACT engine.
    No collectives needed: every core receives the full x plus its row block.
"""

import sys

if "/opt/trn_rl_repo" not in sys.path:
    sys.path.insert(0, "/opt/trn_rl_repo")

import numpy as np

N, D = 8192, 1024
NCORES = 8
ROWS = N // NCORES  # 1024 rows per core
P = 128
NB = 512  # rows of x processed per block iteration
NJ = N // NB  # 16 block iterations
KC = D // P  # 8 k-chunks (contraction over D)
MC = ROWS // P  # 8 local row chunks
NI = NB // P  # 4 n128 chunks per block
MB = 512  # MM1 moving free dim (local rows)
DB = 512  # MM2 moving free dim (feature dim)

_prog_cache = {}


def _build_program():
    import concourse.bass as bass
    import concourse.bacc as bacc
    import concourse.mybir as mybir
    import concourse.tile as tile
    from concourse.masks import make_identity

    f32 = mybir.dt.float32
    f32r = mybir.dt.float32r
    AF = mybir.ActivationFunctionType

    nc = bacc.Bacc(trn_type="TRN2", target_bir_lowering=False, debug=False)
    x_d = nc.dram_tensor("x", [N, D], f32, kind="ExternalInput").ap()
    xloc_d = nc.dram_tensor("xloc", [ROWS, D], f32, kind="ExternalInput").ap()
    out_d = nc.dram_tensor("out", [ROWS, D], f32, kind="ExternalOutput").ap()

    with tile.TileContext(nc) as tc:
        with (
            tc.tile_pool(name="singles", bufs=1) as singles,
            tc.tile_pool(name="xj", bufs=2) as xj_pool,
            tc.tile_pool(name="xjr", bufs=2) as xjr_pool,
            tc.tile_pool(name="xT", bufs=1) as xT_pool,
            tc.tile_pool(name="st", bufs=1) as st_pool,
            tc.tile_pool(name="scr", bufs=2) as scr_pool,
            tc.tile_pool(name="xl", bufs=4) as xl_pool,
            tc.tile_pool(name="small", bufs=4) as small,
            tc.tile_pool(name="outp", bufs=2) as out_pool,
            tc.tile_pool(name="ps_t", bufs=3, space="PSUM") as ps_t,
            tc.tile_pool(name="ps_mm1", bufs=2, space="PSUM") as ps_mm1,
            tc.tile_pool(name="ps_mm2", bufs=2, space="PSUM") as ps_mm2,
        ):
            # make_identity writes via gpsimd; bounce through DVE so matmuls
            # that read the identity never need a gpsimd sync wait (walrus
            # limits sync waits per fused matmult).
            ident_g = singles.tile([P, P], f32)
            make_identity(nc, ident_g)
            ident = singles.tile([P, P], f32)
            nc.vector.tensor_copy(out=ident, in_=ident_g)
            ident_r = singles.tile([P, P], f32r)
            nc.vector.tensor_copy(out=ident_r, in_=ident)

            xlocT = singles.tile([P, KC, ROWS], f32r)  # x_loc^T, resident
            invloc = singles.tile([P, MC], f32)  # 1/||x_loc row||
            p_acc = singles.tile([P, MC, D], f32)  # MM2 accumulator

            # ---- setup: transpose x_loc into xlocT, compute invloc ----
            ssq_l = small.tile([P, MC], f32)
            for i in range(MC):
                xl = xl_pool.tile([P, D], f32, tag="xl")
                nc.sync.dma_start(out=xl, in_=xloc_d[i * P : (i + 1) * P, :])
                sq = scr_pool.tile([P, D], f32, tag="sq")
                nc.scalar.activation(
                    out=sq, in_=xl, func=AF.Square, accum_out=ssq_l[:, i : i + 1]
                )
                for j2 in range(KC // 4):
                    pst = ps_t.tile([P, 4 * P], f32)
                    for j3 in range(4):
                        j = j2 * 4 + j3
                        nc.tensor.transpose(
                            pst[:, j3 * P : (j3 + 1) * P],
                            xl[:, j * P : (j + 1) * P],
                            ident,
                        )
                    for j3 in range(4):
                        j = j2 * 4 + j3
                        nc.vector.tensor_copy(
                            out=xlocT[:, j, i * P : (i + 1) * P],
                            in_=pst[:, j3 * P : (j3 + 1) * P],
                        )
            nrm_l = small.tile([P, MC], f32)
            nc.scalar.activation(out=nrm_l, in_=ssq_l, func=AF.Sqrt)
            nc.vector.reciprocal(invloc, nrm_l)

            # ---- main loop over blocks of NB rows of x ----
            for jb in range(NJ):
                xj = xj_pool.tile([P, NI, D], f32)
                src = x_d[jb * NB : (jb + 1) * NB, :].rearrange(
                    "(i p) d -> p i d", p=P
                )
                nc.sync.dma_start(out=xj, in_=src)

                # row norms of this block + f32r rounding
                xjr = xjr_pool.tile([P, NI, D], f32r)
                ssq = small.tile([P, NI], f32)
                for i in range(NI):
                    sq = scr_pool.tile([P, D], f32, tag="sq")
                    nc.scalar.activation(
                        out=sq,
                        in_=xj[:, i, :],
                        func=AF.Square,
                        accum_out=ssq[:, i : i + 1],
                    )
                    nc.vector.tensor_copy(out=xjr[:, i, :], in_=xj[:, i, :])
                nrm = small.tile([P, NI], f32)
                nc.scalar.activation(out=nrm, in_=ssq, func=AF.Sqrt)
                inv_j = small.tile([P, NI], f32)
                nc.vector.reciprocal(inv_j, nrm)

                # transpose block into xT [P(k), KC, NB(n)]
                xT = xT_pool.tile([P, KC, NB], f32r)
                for j in range(KC):
                    pst = ps_t.tile([P, NB], f32)
                    for i in range(NI):
                        nc.tensor.transpose(
                            pst[:, i * P : (i + 1) * P].bitcast(f32r),
                            xjr[:, i, j * P : (j + 1) * P],
                            ident_r,
                        )
                    nc.vector.tensor_copy(out=xT[:, j, :], in_=pst.bitcast(f32r))

                # MM1: st[n, m] = inv_j[n] * sum_k x[n,k] xloc[m,k]
                st = st_pool.tile([P, NI, ROWS], f32r)
                for i in range(NI):
                    for mb in range(ROWS // MB):
                        ps1 = ps_mm1.tile([P, MB], f32)
                        for k in range(KC):
                            nc.tensor.matmul(
                                ps1,
                                xT[:, k, i * P : (i + 1) * P],
                                xlocT[:, k, mb * MB : (mb + 1) * MB],
                                start=(k == 0),
                                stop=(k == KC - 1),
                            )
                        nc.scalar.activation(
                            out=st[:, i, mb * MB : (mb + 1) * MB],
                            in_=ps1,
                            func=AF.Copy,
                            scale=inv_j[:, i : i + 1],
                        )

                # MM2: p_acc[m, d] += sum_n st[n, m] x[n, d]
                for mc in range(MC):
                    for db in range(D // DB):
                        ps2 = ps_mm2.tile([P, DB], f32)
                        for i in range(NI):
                            nc.tensor.matmul(
                                ps2,
                                st[:, i, mc * P : (mc + 1) * P],
                                xjr[:, i, db * DB : (db + 1) * DB],
                                start=(i == 0),
                                stop=(i == NI - 1),
                            )
                        dst = p_acc[:, mc, db * DB : (db + 1) * DB]
                        if jb == 0:
                            nc.vector.tensor_copy(out=dst, in_=ps2)
                        else:
                            nc.vector.tensor_add(out=dst, in0=dst, in1=ps2)

            # ---- final fused scale + sigmoid, write out ----
            for mc in range(MC):
                ot = out_pool.tile([P, D], f32)
                nc.scalar.activation(
                    out=ot,
                    in_=p_acc[:, mc, :],
                    func=AF.Sigmoid,
                    scale=invloc[:, mc : mc + 1],
                )
                nc.sync.dma_start(out=out_d[mc * P : (mc + 1) * P, :], in_=ot)

    nc.compile()
    return nc


def get_program():
    if "nc" not in _prog_cache:
        _prog_cache["nc"] = _build_program()
    return _prog_cache["nc"]


def kernel(x: np.ndarray, W: np.ndarray, _collect=None) -> np.ndarray:
    """Full-input / full-output entry point. W is an unused declared param."""
    from concourse.bass_utils import run_bass_kernel_spmd

    nc = get_program()
    x = np.ascontiguousarray(np.asarray(x, dtype=np.float32))
    in_maps = [
        {"x": x, "xloc": x[c * ROWS : (c + 1) * ROWS]} for c in range(NCORES)
    ]
    res = run_bass_kernel_spmd(
        nc, in_maps, list(range(NCORES)), trace=bool(_collect is not None)
    )
    if _collect is not None:
        _collect["results"] = res
    return np.concatenate([res.results[c]["out"] for c in range(NCORES)], axis=0)


if __name__ == "__main__":
    get_program()
    print("program built OK")



# revision 4
# speedup vs baseline: 2.7771x; 2.7771x over previous
"""CosineAttention Trainium2 kernel.

reference:
    xn  = x / max(||x_i||, eps)        # row-normalize
    sim = xn @ xn.T                    # [N, N]
    out = sigmoid(sim @ x)             # [N, D]

Key identity (matmul associativity):
    sim @ x = xn @ (xn^T @ x) = xn @ A,   A = xn^T x  [D, D]
which is O(N D^2) instead of O(N^2 D): 8x less compute (N/D = 8).
Further, A = B^T B with B = x / sqrt(||x_i||)  (symmetric PSD form), so each
core only materializes one scaled copy B of its row block, and
    out = sigmoid( diag(1/sqrt(||x_i||)) (B @ A) ).

Distribution (8 cores, 1-D row-parallel):
    Core c owns ROWS = N/8 rows.  It computes the partial A_c = B_c^T B_c
    [D, D] from its block alone (contraction over rows lies on the partition
    axis -- no transposes needed), AllReduduces A across the 8 cores (4 MB),
    then computes out_c = sigmoid(sinv * (B_c @ A)) via PE with B_c^T built
    by on-chip transposes while the collective is in flight.
    f32r (TF32-like) matmuls run at full PE rate (~1e-3 rel error).
"""

import sys

if "/opt/trn_rl_repo" not in sys.path:
    sys.path.insert(0, "/opt/trn_rl_repo")

import numpy as np

N, D = 8192, 1024
NCORES = 8
ROWS = N // NCORES  # 1024 rows per core
P = 128
MC = ROWS // P  # 8 row chunks per core
KC = D // P  # 8 k chunks (contraction dim of MM2 / output rows of A)
DB = 512  # moving free dim for both matmuls
ND = D // DB  # 2 d-halves

_prog_cache = {}


def _build_program():
    import concourse.bass as bass
    import concourse.bacc as bacc
    import concourse.mybir as mybir
    import concourse.tile as tile
    from concourse.masks import make_identity

    f32 = mybir.dt.float32
    f32r = mybir.dt.float32r
    AF = mybir.ActivationFunctionType

    nc = bacc.Bacc(
        trn_type="TRN2", target_bir_lowering=False, debug=False, num_devices=NCORES
    )
    xloc_d = nc.dram_tensor("xloc", [ROWS, D], f32, kind="ExternalInput").ap()
    out_d = nc.dram_tensor("out", [ROWS, D], f32, kind="ExternalOutput").ap()

    with tile.TileContext(nc) as tc:
        with (
            tc.tile_pool(name="singles", bufs=1) as singles,
            tc.tile_pool(name="xl", bufs=3) as xl_pool,
            tc.tile_pool(name="scr", bufs=2) as scr_pool,
            tc.tile_pool(name="ast", bufs=3) as ast_pool,
            tc.tile_pool(name="asb", bufs=2) as asb_pool,
            tc.tile_pool(name="outp", bufs=3) as out_pool,
            tc.tile_pool(name="small", bufs=4) as small,
            tc.tile_pool(name="dram", bufs=2, space="DRAM") as dram,
            tc.tile_pool(name="ps_t", bufs=2, space="PSUM") as ps_t,
            tc.tile_pool(name="ps_a", bufs=3, space="PSUM") as ps_a,
            tc.tile_pool(name="ps_o", bufs=3, space="PSUM") as ps_o,
        ):
            # identity for PE transposes; bounce via DVE so matmul readers
            # never wait on gpsimd (walrus sync-wait limit).
            ident_g = singles.tile([P, P], f32)
            make_identity(nc, ident_g)
            ident_r = singles.tile([P, P], f32r)
            nc.vector.tensor_copy(out=ident_r, in_=ident_g)

            B = singles.tile([P, MC, D], f32r)  # x_loc rows scaled by 1/sqrt(nrm)
            BT = singles.tile([P, KC, ROWS], f32r)  # B^T (k on partitions)
            sinv = singles.tile([P, MC], f32)  # 1/sqrt(||row||)

            a_part = dram.tile([D, D], f32r)  # local partial A_c
            a_red = dram.tile([D, D], f32r)  # allreduced A

            # ---- load x_loc, compute norms, scale into B ----
            for rc in range(MC):
                xl = xl_pool.tile([P, D], f32, tag="xl")
                nc.sync.dma_start(out=xl, in_=xloc_d[rc * P : (rc + 1) * P, :])
                sq = scr_pool.tile([P, D], f32, tag="sq")
                ssq = small.tile([P, 1], f32, tag="ssq")
                nc.scalar.activation(
                    out=sq, in_=xl, func=AF.Square, accum_out=ssq
                )
                nrm = small.tile([P, 1], f32, tag="nrm")
                nc.scalar.activation(out=nrm, in_=ssq, func=AF.Sqrt)
                inv = small.tile([P, 1], f32, tag="inv")
                nc.vector.reciprocal(inv, nrm)
                nc.scalar.activation(
                    out=sinv[:, rc : rc + 1], in_=inv, func=AF.Sqrt
                )
                nc.vector.tensor_scalar_mul(
                    out=B[:, rc, :], in0=xl, scalar1=sinv[:, rc : rc + 1]
                )

            # ---- MM-A: A_c[k, d] = sum_r B[r, k] B[r, d]  (r on partitions) ----
            for kc in range(KC):
                for dh in range(ND):
                    ps = ps_a.tile([P, DB], f32)
                    for rc in range(MC):
                        nc.tensor.matmul(
                            ps,
                            B[:, rc, kc * P : (kc + 1) * P],
                            B[:, rc, dh * DB : (dh + 1) * DB],
                            start=(rc == 0),
                            stop=(rc == MC - 1),
                        )
                    a_st = ast_pool.tile([P, DB], f32r, tag="ast")
                    nc.vector.tensor_copy(out=a_st, in_=ps)
                    nc.sync.dma_start(
                        out=a_part[kc * P : (kc + 1) * P, dh * DB : (dh + 1) * DB],
                        in_=a_st,
                    )

            # ---- AllReduce A across the 8 cores (DRAM -> DRAM) ----
            nc.gpsimd.collective_compute(
                "AllReduce",
                mybir.AluOpType.add,
                replica_groups=[list(range(NCORES))],
                ins=[a_part[:].opt()],
                outs=[a_red[:].opt()],
            )

            # ---- build B^T on PE while the collective is in flight ----
            for rc in range(MC):
                for kg in range(KC // 4):
                    pst = ps_t.tile([P, 4 * P], f32)
                    for j3 in range(4):
                        kc = kg * 4 + j3
                        nc.tensor.transpose(
                            pst[:, j3 * P : (j3 + 1) * P].bitcast(f32r),
                            B[:, rc, kc * P : (kc + 1) * P],
                            ident_r,
                        )
                    nc.vector.tensor_copy(
                        out=BT[:, kg * 4 : (kg + 1) * 4, rc * P : (rc + 1) * P],
                        in_=pst.bitcast(f32r).rearrange("p (k q) -> p k q", k=4),
                    )

            # ---- MM2: out = sigmoid(sinv * (B @ A)), d-half at a time ----
            for dh in range(ND):
                a_sb = asb_pool.tile([P, KC, DB], f32r, tag="asb")
                nc.sync.dma_start(
                    out=a_sb,
                    in_=a_red[:, dh * DB : (dh + 1) * DB].rearrange(
                        "(kc p) d -> p kc d", p=P
                    ),
                )
                for mc in range(MC):
                    ps2 = ps_o.tile([P, DB], f32)
                    for kc in range(KC):
                        nc.tensor.matmul(
                            ps2,
                            BT[:, kc, mc * P : (mc + 1) * P],
                            a_sb[:, kc, :],
                            start=(kc == 0),
                            stop=(kc == KC - 1),
                        )
                    ot = out_pool.tile([P, DB], f32, tag="ot")
                    nc.scalar.activation(
                        out=ot,
                        in_=ps2,
                        func=AF.Sigmoid,
                        scale=sinv[:, mc : mc + 1],
                    )
                    nc.sync.dma_start(
                        out=out_d[mc * P : (mc + 1) * P, dh * DB : (dh + 1) * DB],
                        in_=ot,
                    )

    nc.compile()
    return nc


def get_program():
    if "nc" not in _prog_cache:
        _prog_cache["nc"] = _build_program()
    return _prog_cache["nc"]


def kernel(x: np.ndarray, W: np.ndarray, _collect=None) -> np.ndarray:
    """Full-input / full-output entry point. W is an unused declared param."""
    from concourse.bass_utils import run_bass_kernel_spmd

    nc = get_program()
    x = np.ascontiguousarray(np.asarray(x, dtype=np.float32))
    in_maps = [{"xloc": x[c * ROWS : (c + 1) * ROWS]} for c in range(NCORES)]
    res = run_bass_kernel_spmd(
        nc, in_maps, list(range(NCORES)), trace=bool(_collect is not None)
    )
    if _collect is not None:
        _collect["results"] = res
    return np.concatenate([res.results[c]["out"] for c in range(NCORES)], axis=0)


if __name__ == "__main__":
    get_program()
    print("program built OK")


# revision 7
# speedup vs baseline: 3.5571x; 1.2809x over previous
"""CosineAttention Trainium2 kernel.

reference:
    xn  = x / max(||x_i||, eps)        # row-normalize
    sim = xn @ xn.T                    # [N, N]
    out = sigmoid(sim @ x)             # [N, D]

Key identity (matmul associativity):
    sim @ x = xn @ (xn^T @ x) = xn @ A,   A = xn^T x  [D, D]
which is O(N D^2) instead of O(N^2 D): 8x less compute (N/D = 8).
Further, A = B^T B with B = x / sqrt(||x_i||)  (symmetric PSD form), so each
core only materializes one scaled copy B of its row block, and
    out = sigmoid( diag(1/sqrt(||x_i||)) (B @ A) ).

Distribution (8 cores, 1-D row-parallel):
    Core c owns ROWS = N/8 rows.  It computes the partial A_c = B_c^T B_c
    [D, D] from its block (contraction over rows lies on the partition axis
    -- no transposes needed), AllReduces A, then computes
    out_c = sigmoid(sinv * (B_c @ A)).

Collective pipelining: A is split into two column halves, each AllReduced
in fp16 (1 MB wire each) as soon as its MM-A half finishes; B^T transposes
and MM2 on half 0 overlap the AllReduce of half 1.  MM-A runs in f32r
(full PE rate); MM2 runs in fp16 (also full rate, inputs already rounded
by the fp16 collective).
"""

import sys

if "/opt/trn_rl_repo" not in sys.path:
    sys.path.insert(0, "/opt/trn_rl_repo")

import numpy as np

N, D = 8192, 1024
NCORES = 8
ROWS = N // NCORES  # 1024 rows per core
P = 128
MC = ROWS // P  # 8 row chunks per core
KC = D // P  # 8 k chunks (contraction dim of MM2 / output rows of A)
DB = 512  # moving free dim for both matmuls
ND = D // DB  # 2 d-halves

_prog_cache = {}


def _build_program():
    import concourse.bass as bass
    import concourse.bacc as bacc
    import concourse.mybir as mybir
    import concourse.tile as tile
    from concourse.masks import make_identity

    f32 = mybir.dt.float32
    f32r = mybir.dt.float32r
    f16 = mybir.dt.float16
    AF = mybir.ActivationFunctionType

    nc = bacc.Bacc(
        trn_type="TRN2", target_bir_lowering=False, debug=False, num_devices=NCORES
    )
    xloc_d = nc.dram_tensor("xloc", [ROWS, D], f32, kind="ExternalInput").ap()
    out_d = nc.dram_tensor("out", [ROWS, D], f32, kind="ExternalOutput").ap()

    with tile.TileContext(nc) as tc:
        with (
            tc.tile_pool(name="singles", bufs=1) as singles,
            tc.tile_pool(name="xl", bufs=3) as xl_pool,
            tc.tile_pool(name="scr", bufs=2) as scr_pool,
            tc.tile_pool(name="ast", bufs=4) as ast_pool,
            tc.tile_pool(name="asb", bufs=2) as asb_pool,
            tc.tile_pool(name="outp", bufs=3) as out_pool,
            tc.tile_pool(name="small", bufs=4) as small,
            tc.tile_pool(name="dram", bufs=2, space="DRAM") as dram,
            tc.tile_pool(name="ps_t", bufs=2, space="PSUM") as ps_t,
            tc.tile_pool(name="ps_a", bufs=3, space="PSUM") as ps_a,
            tc.tile_pool(name="ps_o", bufs=3, space="PSUM") as ps_o,
        ):
            # identity for PE transposes; bounce via DVE so matmul readers
            # never wait on gpsimd (walrus sync-wait limit).
            ident_g = singles.tile([P, P], f32)
            make_identity(nc, ident_g)
            ident_r = singles.tile([P, P], f32r)
            nc.vector.tensor_copy(out=ident_r, in_=ident_g)

            B = singles.tile([P, MC, D], f32r)  # x_loc rows scaled by 1/sqrt(nrm)
            BT = singles.tile([P, KC, ROWS], f16)  # B^T (k on partitions)
            sinv = singles.tile([P, MC], f32)  # 1/sqrt(||row||)

            # per d-half bounce buffers for the column-split AllReduce
            a_part = [
                dram.tile([D, DB], f16, tag=f"ap{h}", name=f"a_part{h}")
                for h in range(ND)
            ]
            a_red = [
                dram.tile(
                    [D, DB], f16, tag=f"ar{h}", name=f"a_red{h}", addr_space="Shared"
                )
                for h in range(ND)
            ]

            # ---- load x_loc, compute norms, scale into B ----
            for rc in range(MC):
                xl = xl_pool.tile([P, D], f32, tag="xl")
                nc.sync.dma_start(out=xl, in_=xloc_d[rc * P : (rc + 1) * P, :])
                sq = scr_pool.tile([P, D], f32, tag="sq")
                ssq = small.tile([P, 1], f32, tag="ssq")
                nc.scalar.activation(out=sq, in_=xl, func=AF.Square, accum_out=ssq)
                nrm = small.tile([P, 1], f32, tag="nrm")
                nc.scalar.activation(out=nrm, in_=ssq, func=AF.Sqrt)
                inv = small.tile([P, 1], f32, tag="inv")
                nc.vector.reciprocal(inv, nrm)
                nc.scalar.activation(
                    out=sinv[:, rc : rc + 1], in_=inv, func=AF.Sqrt
                )
                nc.vector.tensor_scalar_mul(
                    out=B[:, rc, :], in0=xl, scalar1=sinv[:, rc : rc + 1]
                )

            # ---- MM-A + per-half AllReduce ----
            # A_c[k, d] = sum_r B[r, k] B[r, d]  (r on partitions)
            for dh in range(ND):
                for kc in range(KC):
                    ps = ps_a.tile([P, DB], f32)
                    for rc in range(MC):
                        nc.tensor.matmul(
                            ps,
                            B[:, rc, kc * P : (kc + 1) * P],
                            B[:, rc, dh * DB : (dh + 1) * DB],
                            start=(rc == 0),
                            stop=(rc == MC - 1),
                        )
                    a_st = ast_pool.tile([P, DB], f16, tag="ast")
                    nc.vector.tensor_copy(out=a_st, in_=ps)
                    nc.sync.dma_start(
                        out=a_part[dh][kc * P : (kc + 1) * P, :], in_=a_st
                    )
                nc.gpsimd.collective_compute(
                    "AllReduce",
                    mybir.AluOpType.add,
                    replica_groups=[list(range(NCORES))],
                    ins=[a_part[dh][:].opt()],
                    outs=[a_red[dh][:].opt()],
                )

            # ---- build B^T on PE while the collectives are in flight ----
            for rc in range(MC):
                for kg in range(KC // 4):
                    pst = ps_t.tile([P, 4 * P], f32)
                    for j3 in range(4):
                        kc = kg * 4 + j3
                        nc.tensor.transpose(
                            pst[:, j3 * P : (j3 + 1) * P].bitcast(f32r),
                            B[:, rc, kc * P : (kc + 1) * P],
                            ident_r,
                        )
                    nc.vector.tensor_copy(
                        out=BT[:, kg * 4 : (kg + 1) * 4, rc * P : (rc + 1) * P],
                        in_=pst.rearrange("p (k q) -> p k q", k=4),
                    )

            # ---- MM2: out = sigmoid(sinv * (B @ A)), d-half at a time ----
            for dh in range(ND):
                a_sb = asb_pool.tile([P, KC, DB], f16, tag="asb")
                nc.sync.dma_start(
                    out=a_sb,
                    in_=a_red[dh][:].rearrange("(kc p) d -> p kc d", p=P),
                )
                for mc in range(MC):
                    ps2 = ps_o.tile([P, DB], f32)
                    for kc in range(KC):
                        nc.tensor.matmul(
                            ps2,
                            BT[:, kc, mc * P : (mc + 1) * P],
                            a_sb[:, kc, :],
                            start=(kc == 0),
                            stop=(kc == KC - 1),
                        )
                    ot = out_pool.tile([P, DB], f32, tag="ot")
                    nc.scalar.activation(
                        out=ot,
                        in_=ps2,
                        func=AF.Sigmoid,
                        scale=sinv[:, mc : mc + 1],
                    )
                    nc.sync.dma_start(
                        out=out_d[mc * P : (mc + 1) * P, dh * DB : (dh + 1) * DB],
                        in_=ot,
                    )

    nc.compile()
    return nc


def get_program():
    if "nc" not in _prog_cache:
        _prog_cache["nc"] = _build_program()
    return _prog_cache["nc"]


def kernel(x: np.ndarray, W: np.ndarray, _collect=None) -> np.ndarray:
    """Full-input / full-output entry point. W is an unused declared param."""
    from concourse.bass_utils import run_bass_kernel_spmd

    nc = get_program()
    x = np.ascontiguousarray(np.asarray(x, dtype=np.float32))
    in_maps = [{"xloc": x[c * ROWS : (c + 1) * ROWS]} for c in range(NCORES)]
    res = run_bass_kernel_spmd(
        nc, in_maps, list(range(NCORES)), trace=bool(_collect is not None)
    )
    if _collect is not None:
        _collect["results"] = res
    return np.concatenate([res.results[c]["out"] for c in range(NCORES)], axis=0)


if __name__ == "__main__":
    get_program()
    print("program built OK")
